# revision 16
# baseline (speedup 1.0000x reference)
"""C2Q attention kernel for Trainium2 (8 NeuronCores, SPMD over batch).

Computes, for inputs similarity [B=32, C=2048, Q=512] f32 and
qencode [B=32, Q=512, H=1024] f32:

    attn = softmax(similarity, axis=-1)
    out  = einsum('bcq,bqh->bch', attn, qencode)

Sharding: data-parallel over batch, 4 batches per core, no collectives.

v2 design (vs the f32-I/O v1): the baseline was HBM-bound (58.8 MB of
f32 traffic per core ~ 164 us at 358 GB/s). This version:
  * casts both inputs to fp16 on the host and pre-transposes similarity
    to [B, Q, C] so the exp'd tile is already in the matmul's stationary
    [q, c] layout -- no PE transposes, no transpose PSUM traffic;
  * computes the softmax denominator with N=1 matmuls against a ones
    vector (reusing the stationary already loaded for the H matmuls);
  * normalizes in the PSUM->SBUF copies (fp16 out), split ACT/DVE;
  * stores the output as fp16 and upcasts on the host.
HBM traffic drops to 29.4 MB/core (~82 us), below the fp16 PE roofline
of ~109 us, making the kernel compute-bound as intended.
"""

import numpy as np
from contextlib import ExitStack

import concourse.bass as bass
import concourse.tile as tile
from concourse import bacc, mybir
from concourse.bass_utils import run_bass_kernel_spmd

B, C, Q, H = 32, 2048, 512, 1024
N_CORES = 8
BPC = B // N_CORES          # batches per core
P = 128                     # partitions
KQ = Q // P                 # q chunks (contraction tiles)
CB = 1024                   # c columns per group
SG = CB // P                # c-subtiles per group
GPB = C // CB               # groups per batch
NG = BPC * GPB              # groups per core
NH = H // 512               # h psum banks per subtile

F32 = mybir.dt.float32
F16 = mybir.dt.float16

MM_MODE = "fp16"            # kept for test.py compat


def build_nc(act_copies=(0, 3, 6)):
    """act_copies: which of the SG per-group output copies run on ACT
    (the rest run on DVE), balancing the two PSUM-draining engines."""
    nc = bacc.Bacc(None, target_bir_lowering=False)
    simT = nc.dram_tensor("simT", [BPC, Q, C], F16, kind="ExternalInput")
    qe = nc.dram_tensor("qe", [BPC, Q, H], F16, kind="ExternalInput")
    out = nc.dram_tensor("out", [BPC, C, H], F16, kind="ExternalOutput")

    with ExitStack() as ctx:
        tc = ctx.enter_context(tile.TileContext(nc))

        const_pool = ctx.enter_context(tc.tile_pool(name="const", bufs=1))
        ones = const_pool.tile([P, 1], F16)
        nc.vector.memset(ones[:], 1.0)
        warm_w = const_pool.tile([P, P], F16)
        nc.vector.memset(warm_w[:], 0.0)

        qe_pool = ctx.enter_context(tc.tile_pool(name="qe", bufs=2))
        sim_pool = ctx.enter_context(tc.tile_pool(name="simt", bufs=3))
        exp_pool = ctx.enter_context(tc.tile_pool(name="expt", bufs=3))
        out_pool = ctx.enter_context(tc.tile_pool(name="outsb", bufs=3))
        recip_pool = ctx.enter_context(tc.tile_pool(name="recip", bufs=3))
        mm_pool = ctx.enter_context(tc.tile_pool(name="mmps", bufs=3, space="PSUM"))
        den_pool = ctx.enter_context(tc.tile_pool(name="denps", bufs=2, space="PSUM"))

        # ~3.5 us of dummy matmuls ahead of the first real work: they run
        # during the preamble + first DMA + first exp, flipping the PE HAM
        # clock gate to 8/8 (2.4 GHz) before the real matmuls start. The
        # target psum tile is recycled by the real matmuls (start=True
        # clears the bank).
        warm_ps = mm_pool.tile([P, H], F32, name="mm_ps")
        for _ in range(32):
            nc.tensor.matmul(warm_ps[:, 0:P], warm_w[:], warm_w[:])

        qe_tiles = {}

        def load_qe(b, split=False):
            """qe rides the gpsimd DMA ring so it never queues behind the
            similarity loads on the sync ring (v4's 7 us startup stall).
            split=True loads per q-chunk so the first contraction matmul
            only waits on a quarter of the transfer."""
            qe_t = qe_pool.tile([P, KQ * H], F16, name="qe_t")
            dst = qe_t[:].rearrange("p (k h) -> p k h", h=H)
            src = qe[b].rearrange("(k p) h -> p k h", p=P)
            if split:
                # k0 lands in halves so the very first contraction matmul
                # waits on an eighth of the transfer, not all of it
                nc.gpsimd.dma_start(dst[:, 0:1, 0:512], src[:, 0:1, 0:512])
                nc.gpsimd.dma_start(dst[:, 0:1, 512:1024], src[:, 0:1, 512:1024])
                for k in range(1, KQ):
                    nc.gpsimd.dma_start(dst[:, k:k + 1, :], src[:, k:k + 1, :])
            else:
                nc.gpsimd.dma_start(dst, src)
            qe_tiles[b] = qe_t

        def stage_load(b, g, split=False):
            """1 MiB fp16 load of one group's [q, c] similarity block.
            split=True loads per q-chunk so the first exp can start after
            ~a quarter of the transfer (startup latency)."""
            sim_t = sim_pool.tile([P, KQ * CB], F16, name="sim_t")
            src = simT[b, :, g * CB:(g + 1) * CB].rearrange("(k p) c -> p k c", p=P)
            dst = sim_t[:].rearrange("p (k c) -> p k c", c=CB)
            if split:
                for k in range(KQ):
                    nc.sync.dma_start(dst[:, k:k + 1, :], src[:, k:k + 1, :])
            else:
                nc.sync.dma_start(dst, src)
            if b not in qe_tiles:
                load_qe(b, split=split)
            return (b, g, sim_t)

        def stage_exp(st, split=False):
            """exp on ACT; output is already the matmul's stationary [q, c]
            fp16 layout. split=True emits one instruction per q-chunk so the
            first matmuls can start earlier (startup latency)."""
            b, g, sim_t = st
            exp_t = exp_pool.tile([P, KQ * CB], F16, name="exp_t")
            if split:
                for k in range(KQ):
                    nc.scalar.activation(
                        exp_t[:, k * CB:(k + 1) * CB], sim_t[:, k * CB:(k + 1) * CB],
                        mybir.ActivationFunctionType.Exp)
            else:
                nc.scalar.activation(
                    exp_t[:], sim_t[:], mybir.ActivationFunctionType.Exp)
            return (b, g, exp_t)

        def stage_work(st, first=False, last=False):
            """Per subtile: 8 N=512 contraction matmuls + 4 N=1 denominator
            matmuls (same stationaries), reciprocal on DVE, normalization
            fused into the PSUM->SBUF fp16 copies, 0.5 MiB stores.

            first: run all copies on DVE (ACT is still busy with the split
            startup exps; PSUM recycling must not wait on it).
            last: per-subtile stores + final copy on ACT to minimize the
            drain tail after the last matmul."""
            b, g, exp_t = st
            qe_t = qe_tiles[b]
            recip = recip_pool.tile([P, SG], F32, name="recip")
            out_sb = out_pool.tile([P, SG * H], F16, name="out_sb")
            for s in range(SG):
                ps = mm_pool.tile([P, H], F32, name="mm_ps")
                den_ps = den_pool.tile([P, 1], F32, name="den_ps")
                for k in range(KQ):
                    w = exp_t[:, k * CB + s * P: k * CB + (s + 1) * P]
                    st_ = (k == 0)
                    sp = (k == KQ - 1)
                    nc.tensor.matmul(ps[:, 0:512], w, qe_t[:, k * H:k * H + 512],
                                     start=st_, stop=sp)
                    nc.tensor.matmul(ps[:, 512:1024], w, qe_t[:, k * H + 512:(k + 1) * H],
                                     start=st_, stop=sp)
                    nc.tensor.matmul(den_ps[:], w, ones[:],
                                     start=st_, stop=sp)
                    if first and s == 0 and k < KQ - 1:
                        # keep the PE (and its HAM clock gate) busy while
                        # the next exp/qe chunks drip in
                        for _ in range(5):
                            nc.tensor.matmul(warm_ps[:, 0:P], warm_w[:], warm_w[:])
                r = recip[:, s:s + 1]
                nc.vector.reciprocal(r, den_ps[:])
                o = s * H
                on_act = (s in act_copies and not first) or (last and s == SG - 1)
                if on_act:
                    nc.scalar.activation(
                        out_sb[:, o:o + H], ps[:],
                        mybir.ActivationFunctionType.Copy, scale=r)
                else:
                    nc.vector.tensor_scalar_mul(out_sb[:, o:o + H], ps[:], r)
                # Steady-state stores ride the gpsimd DMA ring (parallel to
                # the sync ring carrying the similarity loads), per
                # subtile-pair. The host permutes similarity columns
                # odd/even per 256-block so a pair store writes 4 KiB
                # contiguous per partition (halves DMA packet count). The
                # final group stores per subtile on the (by then idle) sync
                # ring to minimize the drain tail.
                if last and s >= SG - 2:
                    cb = g * CB + (SG - 2) * P
                    t = s % 2
                    nc.sync.dma_start(
                        out[b, cb:cb + 2 * P, :].rearrange(
                            "(p t) h -> p t h", p=P)[:, t:t + 1, :],
                        out_sb[:, s * H:(s + 1) * H].rearrange(
                            "p (t h) -> p t h", h=H),
                    )
                elif s % 2 == 1:
                    cb = g * CB + (s - 1) * P
                    nc.gpsimd.dma_start(
                        out[b, cb:cb + 2 * P, :].rearrange("(p t) h -> p t h", p=P),
                        out_sb[:, (s - 1) * H:(s + 1) * H].rearrange(
                            "p (t h) -> p t h", h=H),
                    )

        # 3-deep software pipeline over groups:
        #   iteration i emits DMA(i), EXP(i-1), WORK(i-2)
        bg = [(b, g) for b in range(BPC) for g in range(GPB)]
        st_load = st_exp = None
        for i in range(NG + 2):
            new_load = stage_load(*bg[i], split=(i == 0)) if i < NG else None
            new_exp = stage_exp(st_load, split=(st_load[0] == 0 and st_load[1] == 0)) \
                if st_load is not None else None
            if st_exp is not None:
                stage_work(st_exp,
                           first=(st_exp[0] == 0 and st_exp[1] == 0),
                           last=(i == NG + 1))
            st_load, st_exp = new_load, new_exp

    nc.finalize()
    return nc


_NC_CACHE = {}


def _get_nc(mode=MM_MODE):
    if mode not in _NC_CACHE:
        _NC_CACHE[mode] = build_nc()
    return _NC_CACHE[mode]


# Odd/even permutation of the C axis per 256-row block: subtile 2j holds
# even output rows, 2j+1 odd rows, so a subtile-pair store writes 4 KiB
# contiguous per partition (rows 2p, 2p+1).
_C_PERM = np.concatenate(
    [blk * 256 + np.r_[np.arange(0, 256, 2), np.arange(1, 256, 2)]
     for blk in range(C // 256)])


def run(similarity, qencode, mode=MM_MODE, **spmd_kwargs):
    nc = _get_nc(mode)
    simT = np.ascontiguousarray(
        np.asarray(similarity, dtype=np.float16).transpose(0, 2, 1)[:, :, _C_PERM])
    qe16 = np.asarray(qencode, dtype=np.float16)
    in_maps = [
        {
            "simT": simT[i * BPC:(i + 1) * BPC],
            "qe": qe16[i * BPC:(i + 1) * BPC],
        }
        for i in range(N_CORES)
    ]
    res = run_bass_kernel_spmd(nc, in_maps, core_ids=list(range(N_CORES)), **spmd_kwargs)
    out = np.concatenate([res.results[i]["out"] for i in range(N_CORES)], axis=0)
    return out.astype(np.float32), res


def kernel(similarity, qencode):
    out, _ = run(similarity, qencode)
    return out
